# revision 1
# baseline (speedup 1.0000x reference)
"""CrossAttention Trainium2 Bass kernel.

Full inputs in, full output out. Data-parallel over batch: 8 batch elements
-> 8 NeuronCores; each core runs the whole cross-attention for one batch
element. Weights are replicated; no collectives.

Per-core computation (transposed domain end-to-end):
  x [512, 4096] (c-major)  -> qT = Wq.T @ x            [512(i), 4096(t)]
  ctx [77, 768]            -> k/v = ctxT.T @ Wk/Wv     [77(j), 512(i)]
  per head h (d=64):  simT = kT_h.T @ qT_h             [77(j), t]
                      es = exp(simT / 8)               (scale fused in ACT)
  rowsums: rs8[h,t] = sum_j es_h[j,t] via 8 accumulating selector matmuls
           into one [8, TC] PSUM tile; recip8 = 1/rs8 (DVE approx, 1 op)
  bc_p [128, TC] = selector matmul broadcast of recip8 rows (2p, 2p+1)
  AV pairs col-tiled: pav_p [128, TC] = [v_2p|v_2p+1].T-style pair bank
  ou[:, p, :] = pav_p * bc_p  (single fused DVE tensor_tensor per pair,
           PSUM x PSUM -> SBUF; this is both the PSUM evac and softmax div)
  y = Wo.T @ ou + bo                                   [512(c), 4096(t)]

All matmuls run in float32r (fp32 read as fp22, 1 PE cycle/row at N=512).
No DRAM round trips for the softmax denominator: rowsum packing and the
partition-broadcast both run on the PE via tiny static selector weights.
"""

import os
import sys

for _p in ("/opt/trn_rl_repo", "/root/.axon_site/_ro/trn_rl_repo"):
    if os.path.isdir(_p) and _p not in sys.path:
        sys.path.insert(0, _p)

import numpy as np

C = 512        # channels / model dim
T = 4096       # tokens (H*W)
S = 77         # context length
DCTX = 768     # context dim
HEADS = 8
DH = 64        # head dim
NT = 8         # token chunks
TC = T // NT   # 512 tokens per chunk
CT = C // 128  # 4 c-tiles
KT = DCTX // 128  # 6 context-dim tiles
NP = HEADS // 2   # 4 head pairs

_BUILT = None


def _build(dbg=False):
    import concourse.mybir as mybir
    import concourse.tile as tile
    from concourse import bacc
    from concourse.masks import make_identity

    f32 = mybir.dt.float32
    f32r = mybir.dt.float32r
    AF = mybir.ActivationFunctionType

    nc = bacc.Bacc("TRN2", target_bir_lowering=False, debug=False, num_devices=8)

    X = nc.dram_tensor("x", [C, T], f32, kind="ExternalInput")
    CTX = nc.dram_tensor("ctx", [S, DCTX], f32, kind="ExternalInput")
    WQ = nc.dram_tensor("wq", [C, C], f32, kind="ExternalInput")
    WK = nc.dram_tensor("wk", [DCTX, C], f32, kind="ExternalInput")
    WV = nc.dram_tensor("wv", [DCTX, C], f32, kind="ExternalInput")
    WO = nc.dram_tensor("wo", [C, C], f32, kind="ExternalInput")
    BO = nc.dram_tensor("bo", [C], f32, kind="ExternalInput")
    Y = nc.dram_tensor("y", [C, T], f32, kind="ExternalOutput")
    if dbg:
        DQ = nc.dram_tensor("dq", [128, CT, TC], f32, kind="ExternalOutput")
        DE = nc.dram_tensor("de", [S, TC], f32, kind="ExternalOutput")
        DRS = nc.dram_tensor("drs", [HEADS, TC], f32, kind="ExternalOutput")
        DRC = nc.dram_tensor("drc", [HEADS, TC], f32, kind="ExternalOutput")
        DBC = nc.dram_tensor("dbc", [128, TC], f32, kind="ExternalOutput")
        DOU = nc.dram_tensor("dou", [128, CT, TC], f32, kind="ExternalOutput")

    with tile.TileContext(nc) as tc:
        with (
            tc.tile_pool(name="static", bufs=1) as st,
            tc.tile_pool(name="xin", bufs=3) as xp,
            tc.tile_pool(name="qt", bufs=2) as qp,
            tc.tile_pool(name="expsim", bufs=6) as ep,
            tc.tile_pool(name="outut", bufs=2) as op_,
            tc.tile_pool(name="rcp", bufs=2) as rp,
            tc.tile_pool(name="bcast", bufs=3) as bp,
            tc.tile_pool(name="ysb", bufs=4) as yp,
        ):
            # ---- static loads, ordered by first consumer: ctx/wk/wv feed the
            # setup projections, wq + x chunk 0 feed the first Q projection;
            # wo/bo are issued inside the loop (first needed one chunk later).
            ctxs = st.tile([S, DCTX], f32, tag="ctxs")
            nc.sync.dma_start(ctxs[:], CTX[:])
            ident = st.tile([128, 128], f32, tag="ident")
            make_identity(nc, ident[:])
            wk = st.tile([128, KT, C], f32r, tag="wk")
            nc.sync.dma_start(wk[:], WK[:].bitcast(f32r).rearrange("(o p) i -> p o i", p=128))
            wv = st.tile([128, KT, C], f32r, tag="wv")
            nc.sync.dma_start(wv[:], WV[:].bitcast(f32r).rearrange("(o p) i -> p o i", p=128))
            wq = st.tile([128, CT, C], f32r, tag="wq")
            nc.sync.dma_start(wq[:], WQ[:].bitcast(f32r).rearrange("(o p) i -> p o i", p=128))
            wo = st.tile([128, CT, C], f32r, tag="wo")
            bo = st.tile([128, CT], f32, tag="bo")

            # selector weights (static), built with affine_select (iota
            # compare, out = compare(iota, 0) ? in_ : fill) in f32 scratch,
            # then CAST into f32r (matmul requires f32r-rounded producers).
            # sel77[j, h, c] = (c == h): rowsum of es_h lands on psum row h
            sel77f = st.tile([S, HEADS, HEADS], f32, tag="sel77f")
            nc.gpsimd.memset(sel77f[:], 0.0)
            nc.gpsimd.affine_select(
                out=sel77f[:], in_=sel77f[:],
                compare_op=mybir.AluOpType.not_equal, fill=1.0,
                base=0, channel_multiplier=0,
                pattern=[[-1, HEADS], [1, HEADS]])  # iota = c - h
            sel77 = st.tile([S, HEADS, HEADS], f32r, tag="sel77")
            nc.vector.tensor_copy(sel77[:], sel77f[:])
            # selbc[j, p, half, c] = (j == 2p + half): bc_p = bcast of rcp8 rows
            selbcf = st.tile([HEADS, NP, 2, DH], f32, tag="selbcf")
            nc.gpsimd.memset(selbcf[:], 0.0)
            nc.gpsimd.affine_select(
                out=selbcf[:], in_=selbcf[:],
                compare_op=mybir.AluOpType.not_equal, fill=1.0,
                base=0, channel_multiplier=1,
                pattern=[[-2, NP], [-1, 2], [0, DH]])  # iota = j - 2p - half
            selbc = st.tile([HEADS, NP, 2, DH], f32r, tag="selbc")
            nc.vector.tensor_copy(selbc[:], selbcf[:])

            # ---- setup: context transpose, K/V projections --------------------
            ctxT = st.tile([128, KT, S], f32r, tag="ctxT")
            ktp = st.tile([128, NP, S], f32r, tag="ktp")    # kT head-pairs
            # vpair[:, p, 0] = [v_2p | 0], vpair[:, p, 1] = [0 | v_2p+1]:
            # zero-padded M=128 stationaries so the AV pair accumulates into
            # one [128, TC] bank without col-tiling (quadrant 3 is invalid).
            vpair = st.tile([S, NP, 2, 128], f32r, tag="vpair")
            nc.gpsimd.memset(vpair[:].bitcast(f32), 0.0)
            with tc.tile_pool(name="ps_setup", bufs=1, space="PSUM") as ps_st:
                for ct in range(KT):
                    tp = ps_st.tile([128, S], f32, tag=f"ctx_t{ct % 2}")
                    nc.tensor.transpose(tp[:], ctxs[:, ct * 128:(ct + 1) * 128], ident[0:S, 0:S])
                    nc.vector.tensor_copy(ctxT[:, ct, :], tp[:])
                kps = ps_st.tile([S, C], f32, tag="kproj")
                vps = ps_st.tile([S, C], f32, tag="vproj")
                for ct in range(KT):
                    nc.tensor.matmul(kps[:], ctxT[:, ct, :], wk[:, ct, :],
                                     start=(ct == 0), stop=(ct == KT - 1))
                for ct in range(KT):
                    nc.tensor.matmul(vps[:], ctxT[:, ct, :], wv[:, ct, :],
                                     start=(ct == 0), stop=(ct == KT - 1))
                ksb = st.tile([S, C], f32, tag="ksb")
                nc.vector.tensor_copy(ksb[:], kps[:])
                for h in range(HEADS):
                    half = h % 2
                    nc.vector.tensor_copy(
                        vpair[:, h // 2, half, half * DH:half * DH + DH],
                        vps[:, h * DH:(h + 1) * DH])
                for h in range(HEADS):
                    tp = ps_st.tile([DH, S], f32, tag=f"k_t{h % 2}")
                    nc.tensor.transpose(tp[:], ksb[:, h * DH:(h + 1) * DH], ident[0:S, 0:S])
                    base = (h % 2) * DH
                    nc.vector.tensor_copy(ktp[base:base + DH, h // 2, :], tp[:])

            with (
                tc.tile_pool(name="ps_gemm", bufs=3, space="PSUM") as ps_g,
                tc.tile_pool(name="ps_sim", bufs=2, space="PSUM") as ps_sim,
                tc.tile_pool(name="ps_av", bufs=2, space="PSUM") as ps_av,
                tc.tile_pool(name="ps_rs", bufs=1, space="PSUM") as ps_rs,
            ):
                # ---- main loop over token chunks -----------------------------
                def oproj_group(t, ou, ct):
                    tsl = slice(t * TC, (t + 1) * TC)
                    py = ps_g.tile([128, TC], f32, tag="pg")
                    for it in range(CT):
                        nc.tensor.matmul(py[:], wo[:, it, ct * 128:(ct + 1) * 128], ou[:, it, :],
                                         start=(it == 0), stop=(it == CT - 1))
                    ys = yp.tile([128, TC], f32, tag="ys")
                    if ct % 2 == 0:
                        nc.scalar.activation(ys[:], py[:], AF.Identity, bias=bo[:, ct:ct + 1])
                    else:
                        nc.vector.tensor_scalar_add(ys[:], py[:], bo[:, ct:ct + 1])
                    nc.sync.dma_start(
                        Y[:].rearrange("(o p) t -> p o t", p=128)[:, ct, tsl], ys[:])

                prev = None
                for t in range(NT):
                    xs = xp.tile([128, CT, TC], f32r, tag="xs")
                    nc.sync.dma_start(
                        xs[:], X[:].bitcast(f32r).rearrange("(o p) t -> p o t", p=128)
                        [:, :, t * TC:(t + 1) * TC])
                    if t == 0:
                        # behind x chunk 0 on the queue: needed a chunk later
                        nc.sync.dma_start(
                            wo[:], WO[:].bitcast(f32r).rearrange("(o p) c -> p o c", p=128))
                        nc.sync.dma_start(bo[:], BO[:].rearrange("(o p) -> p o", p=128))

                    # Q projection -> qT [128, 4, TC] (i on partitions)
                    qt = qp.tile([128, CT, TC], f32r, tag="qt")
                    for it in range(CT):
                        pq = ps_g.tile([128, TC], f32, tag="pg")
                        for ct in range(CT):
                            nc.tensor.matmul(pq[:], wq[:, ct, it * 128:(it + 1) * 128], xs[:, ct, :],
                                             start=(ct == 0), stop=(ct == CT - 1))
                        nc.vector.tensor_copy(qt[:, it, :], pq[:])

                    if dbg and t == 0:
                        nc.sync.dma_start(DQ[:], qt[:].bitcast(f32))

                    # QK^T per head + exp (scale 1/8 fused in ACT); rowsums
                    # accumulate into one [8, TC] psum bank via sel77; AV pairs
                    # col-tiled into one [128, TC] bank; chunk t-1's O
                    # projection groups interleave as PE filler.
                    rs8 = ps_rs.tile([HEADS, TC], f32, tag="rs8")
                    exps = []
                    avps = []
                    ogroups = list(range(CT)) if prev is not None else []

                    def emit_oproj_filler():
                        if ogroups:
                            oproj_group(t - 1, prev[0], ogroups.pop(0))

                    for p in range(NP):
                        # QK pair back-to-back: row-tiled (bases 0/64), runs
                        # concurrently on the PE
                        for half in range(2):
                            h = 2 * p + half
                            base = half * DH
                            psim = ps_sim.tile([128, TC], f32, tag="psim")
                            nc.tensor.matmul(psim[0:S, :], ktp[base:base + DH, p, :],
                                             qt[base:base + DH, p, :])
                            es = ep.tile([S, TC], f32r, tag="exps")
                            nc.scalar.activation(es[:], psim[0:S, :], AF.Exp,
                                                 scale=DH ** -0.5)
                            exps.append(es)
                            if dbg and t == 0 and h == 0:
                                nc.sync.dma_start(DE[:], es[:].bitcast(f32))
                        pav = ps_av.tile([128, TC], f32, tag="pav")
                        avps.append(pav)
                        for half in range(2):
                            h = 2 * p + half
                            # rowsum accumulate + AV pair (zero-padded M=128
                            # stationaries accumulate into one psum bank)
                            nc.tensor.matmul(rs8[:], sel77[:, h, :], exps[h][:],
                                             start=(h == 0), stop=(h == HEADS - 1))
                            nc.tensor.matmul(pav[:], vpair[:, p, half, :],
                                             exps[h][:], start=(half == 0),
                                             stop=(half == 1))
                        emit_oproj_filler()

                    # recip of packed rowsums (single DVE op, ~18-bit accurate)
                    rcp8f = rp.tile([HEADS, TC], f32, tag="rcp8f")
                    nc.vector.reciprocal_approx_fast(rcp8f[:], rs8[:])
                    rcp8 = rp.tile([HEADS, TC], f32r, tag="rcp8")
                    nc.vector.tensor_copy(rcp8[:], rcp8f[:])
                    if dbg and t == 0:
                        nc.sync.dma_start(DRS[:], rs8[:])
                        nc.sync.dma_start(DRC[:], rcp8[:])

                    # broadcast + fused evac/normalize per pair:
                    # bc_p[c,t] = rcp8[2p + c//64, t]; ou[:,p,:] = pav_p * bc_p
                    ou = op_.tile([128, CT, TC], f32r, tag="ou")
                    for p in range(NP):
                        pbc = ps_sim.tile([128, TC], f32, tag="psim")
                        nc.tensor.matmul(pbc[:], selbc[:, p, :, :],
                                         rcp8[:])
                        bcs = bp.tile([128, TC], f32, tag="bcs")
                        nc.scalar.activation(bcs[:], pbc[:], AF.Copy)
                        nc.vector.tensor_tensor(
                            ou[:, p, :], avps[p][:], bcs[:],
                            mybir.AluOpType.mult)
                        if dbg and t == 0 and p == 0:
                            nc.sync.dma_start(DBC[:], bcs[:])
                    if dbg and t == 0:
                        nc.sync.dma_start(DOU[:], ou[:].bitcast(f32))

                    # leftover O-projection groups for chunk t-1
                    while ogroups:
                        emit_oproj_filler()

                    prev = (ou,)

                # drain: O projection of the last chunk
                for ct in range(CT):
                    oproj_group(NT - 1, prev[0], ct)

    nc.compile()
    return nc


def _get_nc():
    global _BUILT
    if _BUILT is None:
        _BUILT = _build()
    return _BUILT


def kernel(x, context, Wq, Wk, Wv, Wo, bo):
    from concourse.bass_utils import run_bass_kernel_spmd

    B = x.shape[0]
    assert B == 8 and x.shape == (8, C, 64, 64)
    nc = _get_nc()
    x = np.ascontiguousarray(np.asarray(x, dtype=np.float32))
    in_maps = [
        {
            "x": x[b].reshape(C, T),
            "ctx": np.ascontiguousarray(np.asarray(context[b], np.float32)),
            "wq": np.asarray(Wq, np.float32),
            "wk": np.asarray(Wk, np.float32),
            "wv": np.asarray(Wv, np.float32),
            "wo": np.asarray(Wo, np.float32),
            "bo": np.asarray(bo, np.float32),
        }
        for b in range(B)
    ]
    res = run_bass_kernel_spmd(nc, in_maps, core_ids=list(range(8)))
    return np.stack([r["y"].reshape(C, 64, 64) for r in res.results]).astype(np.float32)



# revision 3
# speedup vs baseline: 1.5584x; 1.5584x over previous
"""CrossAttention Trainium2 Bass kernel (v3).

Full inputs in, full output out. Data-parallel over batch: 8 batch elements
-> 8 NeuronCores; each core runs the whole cross-attention for one batch
element. Weights are replicated; no collectives.

Per-core computation (transposed domain end-to-end):
  x [512, 4096] (c-major)  -> qT = Wq.T @ x            [512(i), 4096(t)]
  ctx [77, 768]            -> k/v = ctxT.T @ Wk/Wv     [77(j), 512(i)]
  per head h (d=64):  simT = kT_h.T @ qT_h             [77(j), t]
                      es = exp(simT / 8)               (scale fused in ACT)
  pair-broadcast rowsums: for pair p, two accumulating matmuls with
      all-ones selector stationaries (ones_lo: cols 0-63, ones_hi: cols
      64-127) produce psrs_p[c, t] = rowsum_{2p + c//64}[t] directly in
      broadcast layout -- no separate rowsum + partition-broadcast passes.
  bc_p = 1/psrs_p  (DVE reciprocal_approx_fast, PSUM -> SBUF)
  AV pairs: pav_p [128, TC] = zero-padded pair bank (2 accumulating MMs)
  ou[:, p, :] = pav_p * bc_p  (fused DVE tensor_tensor; PSUM evac +
      softmax normalization in one op)
  y = Wo.T @ ou + bo                                   [512(c), 4096(t)]

All matmul operands are bf16 (inputs are cast to bf16 host-side, which
also halves input DMA bytes); PSUM accumulation stays fp32. bf16 gives
every 128-column stationary the FWL fast-weight-load path, cutting the
LDWEIGHTS serialization that dominated the fp32r version (382 ns/matmul
issue rate vs the 213 ns streaming floor). Q-projection PSUM evacuation
runs on ACT; reciprocal + fused normalize on DVE; bias adds split
ACT/DVE; all engine loads stay under the PE's.
"""

import os
import sys

for _p in ("/opt/trn_rl_repo", "/root/.axon_site/_ro/trn_rl_repo"):
    if os.path.isdir(_p) and _p not in sys.path:
        sys.path.insert(0, _p)

import numpy as np
import ml_dtypes

BF16 = ml_dtypes.bfloat16

C = 512        # channels / model dim
T = 4096       # tokens (H*W)
S = 77         # context length
DCTX = 768     # context dim
HEADS = 8
DH = 64        # head dim
NT = 8         # token chunks
TC = T // NT   # 512 tokens per chunk
CT = C // 128  # 4 c-tiles
KT = DCTX // 128  # 6 context-dim tiles
NP = HEADS // 2   # 4 head pairs

_BUILT = None


def _build(dbg=False):
    import concourse.mybir as mybir
    import concourse.tile as tile
    from concourse import bacc
    from concourse.masks import make_identity

    f32 = mybir.dt.float32
    bf16 = mybir.dt.bfloat16
    AF = mybir.ActivationFunctionType

    nc = bacc.Bacc("TRN2", target_bir_lowering=False, debug=False, num_devices=8)

    X = nc.dram_tensor("x", [C, T], bf16, kind="ExternalInput")
    CTX = nc.dram_tensor("ctx", [S, DCTX], bf16, kind="ExternalInput")
    WQ = nc.dram_tensor("wq", [C, C], bf16, kind="ExternalInput")
    WK = nc.dram_tensor("wk", [DCTX, C], bf16, kind="ExternalInput")
    WV = nc.dram_tensor("wv", [DCTX, C], bf16, kind="ExternalInput")
    WO = nc.dram_tensor("wo", [C, C], bf16, kind="ExternalInput")
    BO = nc.dram_tensor("bo", [C], f32, kind="ExternalInput")
    Y = nc.dram_tensor("y", [C, T], f32, kind="ExternalOutput")

    with tile.TileContext(nc) as tc:
        with (
            tc.tile_pool(name="static", bufs=1) as st,
            tc.tile_pool(name="xin", bufs=3) as xp,
            tc.tile_pool(name="qt", bufs=2) as qp,
            tc.tile_pool(name="expsim", bufs=6) as ep,
            tc.tile_pool(name="outut", bufs=2) as op_,
            tc.tile_pool(name="bcast", bufs=3) as bp,
            tc.tile_pool(name="ysb", bufs=4) as yp,
        ):
            # ---- static loads, ordered by first consumer: wq + x chunk 0 feed
            # the first Q projection, ctx/wk/wv feed the setup projections;
            # wo/bo are issued inside the loop (first needed one chunk later).
            wq = st.tile([128, CT, C], bf16, tag="wq")
            nc.sync.dma_start(wq[:], WQ[:].rearrange("(o p) i -> p o i", p=128))
            xs0 = xp.tile([128, CT, TC], bf16, tag="xs")
            nc.sync.dma_start(
                xs0[:], X[:].rearrange("(o p) t -> p o t", p=128)[:, :, 0:TC])
            ctxs = st.tile([S, DCTX], bf16, tag="ctxs")
            nc.sync.dma_start(ctxs[:], CTX[:])
            ident = st.tile([128, 128], bf16, tag="ident")
            make_identity(nc, ident[:])
            wk = st.tile([128, KT, C], bf16, tag="wk")
            nc.sync.dma_start(wk[:], WK[:].rearrange("(o p) i -> p o i", p=128))
            wv = st.tile([128, KT, C], bf16, tag="wv")
            nc.sync.dma_start(wv[:], WV[:].rearrange("(o p) i -> p o i", p=128))
            wo = st.tile([128, CT, C], bf16, tag="wo")
            bo = st.tile([128, CT], f32, tag="bo")

            # all-ones selector stationaries (bf16): pair-broadcast rowsums.
            # ones_lo[j, c] = (c < 64), ones_hi[j, c] = (c >= 64)
            ones_lo = st.tile([S, 128], bf16, tag="ones_lo")
            nc.gpsimd.memset(ones_lo[:], 0.0)
            nc.gpsimd.memset(ones_lo[:, 0:DH], 1.0)
            ones_hi = st.tile([S, 128], bf16, tag="ones_hi")
            nc.gpsimd.memset(ones_hi[:], 0.0)
            nc.gpsimd.memset(ones_hi[:, DH:128], 1.0)

            # ---- setup: context transpose, K/V projections --------------------
            ctxT = st.tile([128, KT, S], bf16, tag="ctxT")
            ktp = st.tile([128, NP, S], bf16, tag="ktp")    # kT head-pairs
            # vpair[:, p, 0] = [v_2p | 0], vpair[:, p, 1] = [0 | v_2p+1]:
            # zero-padded M=128 stationaries so the AV pair accumulates into
            # one [128, TC] bank without col-tiling (quadrant 3 is invalid).
            vpair = st.tile([S, NP, 2, 128], bf16, tag="vpair")
            nc.gpsimd.memset(vpair[:], 0.0)
            with tc.tile_pool(name="ps_setup", bufs=1, space="PSUM") as ps_st:
                for ct in range(KT):
                    tp = ps_st.tile([128, S], bf16, tag=f"ctx_t{ct % 2}")
                    nc.tensor.transpose(tp[:], ctxs[:, ct * 128:(ct + 1) * 128], ident[0:S, 0:S])
                    nc.vector.tensor_copy(ctxT[:, ct, :], tp[:])
                kps = ps_st.tile([S, C], f32, tag="kproj")
                vps = ps_st.tile([S, C], f32, tag="vproj")
                for ct in range(KT):
                    nc.tensor.matmul(kps[:], ctxT[:, ct, :], wk[:, ct, :],
                                     start=(ct == 0), stop=(ct == KT - 1))
                for ct in range(KT):
                    nc.tensor.matmul(vps[:], ctxT[:, ct, :], wv[:, ct, :],
                                     start=(ct == 0), stop=(ct == KT - 1))
                ksb = st.tile([S, C], bf16, tag="ksb")
                nc.vector.tensor_copy(ksb[:], kps[:])
                for h in range(HEADS):
                    half = h % 2
                    nc.vector.tensor_copy(
                        vpair[:, h // 2, half, half * DH:half * DH + DH],
                        vps[:, h * DH:(h + 1) * DH])
                for h in range(HEADS):
                    tp = ps_st.tile([DH, S], bf16, tag=f"k_t{h % 2}")
                    nc.tensor.transpose(tp[:], ksb[:, h * DH:(h + 1) * DH], ident[0:S, 0:S])
                    base = (h % 2) * DH
                    nc.vector.tensor_copy(ktp[base:base + DH, h // 2, :], tp[:])

            with (
                tc.tile_pool(name="ps_gemm", bufs=2, space="PSUM") as ps_g,
                tc.tile_pool(name="ps_sim", bufs=2, space="PSUM") as ps_sim,
                tc.tile_pool(name="ps_av", bufs=2, space="PSUM") as ps_av,
                tc.tile_pool(name="ps_rs", bufs=2, space="PSUM") as ps_rs,
            ):
                # ---- main loop over token chunks -----------------------------
                def oproj_group(t, ou, ct):
                    tsl = slice(t * TC, (t + 1) * TC)
                    py = ps_g.tile([128, TC], f32, tag="pg")
                    for it in range(CT):
                        nc.tensor.matmul(py[:], wo[:, it, ct * 128:(ct + 1) * 128], ou[:, it, :],
                                         start=(it == 0), stop=(it == CT - 1))
                    ys = yp.tile([128, TC], f32, tag="ys")
                    if ct % 2 == 0:
                        nc.scalar.activation(ys[:], py[:], AF.Identity, bias=bo[:, ct:ct + 1])
                    else:
                        nc.vector.tensor_scalar_add(ys[:], py[:], bo[:, ct:ct + 1])
                    nc.sync.dma_start(
                        Y[:].rearrange("(o p) t -> p o t", p=128)[:, ct, tsl], ys[:])

                prev = None
                for t in range(NT):
                    if t == 0:
                        xs = xs0
                    else:
                        xs = xp.tile([128, CT, TC], bf16, tag="xs")
                        nc.sync.dma_start(
                            xs[:], X[:].rearrange("(o p) t -> p o t", p=128)
                            [:, :, t * TC:(t + 1) * TC])
                    if t == 0:
                        # behind x chunk 0 on the queue: needed a chunk later
                        nc.sync.dma_start(
                            wo[:], WO[:].rearrange("(o p) c -> p o c", p=128))
                        nc.sync.dma_start(bo[:], BO[:].rearrange("(o p) -> p o", p=128))

                    # Q projection -> qT [128, 4, TC] (i on partitions);
                    # PSUM evac on ACT (DVE carries recip + fused normalize)
                    qt = qp.tile([128, CT, TC], bf16, tag="qt")
                    for it in range(CT):
                        pq = ps_g.tile([128, TC], f32, tag="pg")
                        for ct in range(CT):
                            nc.tensor.matmul(pq[:], wq[:, ct, it * 128:(it + 1) * 128], xs[:, ct, :],
                                             start=(ct == 0), stop=(ct == CT - 1))
                        nc.scalar.activation(qt[:, it, :], pq[:], AF.Copy)

                    # QK^T per head + exp (scale 1/8 fused in ACT); pair-
                    # broadcast rowsums + AV pairs accumulate per pair; chunk
                    # t-1's O projection groups interleave as PE filler.
                    exps = []
                    pairs = []
                    ogroups = list(range(CT)) if prev is not None else []

                    def emit_oproj_filler():
                        if ogroups:
                            oproj_group(t - 1, prev[0], ogroups.pop(0))

                    for p in range(NP):
                        # QK pair back-to-back: row-tiled (bases 0/64), runs
                        # concurrently on the PE
                        for half in range(2):
                            h = 2 * p + half
                            base = half * DH
                            psim = ps_sim.tile([128, TC], f32, tag="psim")
                            nc.tensor.matmul(psim[0:S, :], ktp[base:base + DH, p, :],
                                             qt[base:base + DH, p, :])
                            es = ep.tile([S, TC], bf16, tag="exps")
                            nc.scalar.activation(es[:], psim[0:S, :], AF.Exp,
                                                 scale=DH ** -0.5)
                            exps.append(es)
                        pav = ps_av.tile([128, TC], f32, tag="pav")
                        prs = ps_rs.tile([128, TC], f32, tag="prs")
                        pairs.append((pav, prs))
                        for half in range(2):
                            h = 2 * p + half
                            # pair-broadcast rowsum accumulate + AV pair
                            # (zero-padded M=128 stationaries accumulate into
                            # one psum bank)
                            nc.tensor.matmul(prs[:], ones_lo[:] if half == 0 else ones_hi[:],
                                             exps[h][:], start=(half == 0),
                                             stop=(half == 1))
                            nc.tensor.matmul(pav[:], vpair[:, p, half, :],
                                             exps[h][:], start=(half == 0),
                                             stop=(half == 1))
                        emit_oproj_filler()

                    # per-pair: bc_p = 1/psrs_p (PSUM -> SBUF, one DVE op),
                    # then fused evac/normalize: ou[:,p,:] = pav_p * bc_p
                    ou = op_.tile([128, CT, TC], bf16, tag="ou")
                    for p in range(NP):
                        pav, prs = pairs[p]
                        bcs = bp.tile([128, TC], f32, tag="bcs")
                        nc.vector.reciprocal_approx_fast(bcs[:], prs[:])
                        nc.vector.tensor_tensor(
                            ou[:, p, :], pav[:], bcs[:],
                            mybir.AluOpType.mult)

                    # leftover O-projection groups for chunk t-1
                    while ogroups:
                        emit_oproj_filler()

                    prev = (ou,)

                # drain: O projection of the last chunk
                for ct in range(CT):
                    oproj_group(NT - 1, prev[0], ct)

    nc.compile()
    return nc


def _get_nc():
    global _BUILT
    if _BUILT is None:
        _BUILT = _build()
    return _BUILT


def kernel(x, context, Wq, Wk, Wv, Wo, bo):
    from concourse.bass_utils import run_bass_kernel_spmd

    B = x.shape[0]
    assert B == 8 and x.shape == (8, C, 64, 64)
    nc = _get_nc()
    x8 = np.asarray(x, dtype=np.float32).reshape(B, C, T).astype(BF16)
    ctx8 = np.asarray(context, dtype=np.float32).astype(BF16)
    wq8 = np.asarray(Wq, np.float32).astype(BF16)
    wk8 = np.asarray(Wk, np.float32).astype(BF16)
    wv8 = np.asarray(Wv, np.float32).astype(BF16)
    wo8 = np.asarray(Wo, np.float32).astype(BF16)
    in_maps = [
        {
            "x": np.ascontiguousarray(x8[b]),
            "ctx": np.ascontiguousarray(ctx8[b]),
            "wq": wq8,
            "wk": wk8,
            "wv": wv8,
            "wo": wo8,
            "bo": np.asarray(bo, np.float32),
        }
        for b in range(8)
    ]
    res = run_bass_kernel_spmd(nc, in_maps, core_ids=list(range(8)))
    return np.stack([r["y"].reshape(C, 64, 64) for r in res.results]).astype(np.float32)


# revision 5
# speedup vs baseline: 1.6116x; 1.0342x over previous
"""CrossAttention Trainium2 Bass kernel (v4).

Full inputs in, full output out. Data-parallel over batch: 8 batch elements
-> 8 NeuronCores; each core runs the whole cross-attention for one batch
element. Weights are replicated; no collectives.

Per-core computation (transposed domain end-to-end):
  x [512, 4096] (c-major)  -> qT = Wq.T @ x            [512(i), 4096(t)]
  ctx [77, 768]            -> k/v = ctxT.T @ Wk/Wv     [77(j), 512(i)]
  per head pair p: simT pair in one 2-bank PSUM tile   [77(j), 2, t]
      (the two QK matmuls are row-tiled at partition bases 0/64 and run
      concurrently on the PE); one paired ACT exp evacuates both halves.
  pair-broadcast rowsums: two accumulating matmuls with all-ones selector
      stationaries (ones_lo: cols 0-63, ones_hi: cols 64-127) produce
      psrs_p[c, t] = rowsum_{2p + c//64}[t] directly in broadcast layout.
  bc_p = 1/psrs_p  (DVE reciprocal_approx_fast, PSUM -> SBUF)
  AV pairs: pav_p [128, TC] = zero-padded pair bank (2 accumulating MMs)
  ou[:, p, :] = pav_p * bc_p  (fused DVE tensor_tensor; PSUM evac +
      softmax normalization in one op)
  y = Wo.T @ ou + bo                                   [512(c), 4096(t)]

All matmul operands are bf16 (inputs are cast to bf16 host-side, halving
input DMA bytes; the output is written bf16 and upcast host-side). bf16
gives every 128-column stationary the FWL fast-weight-load path, keeping
the matmul issue rate at the 213 ns streaming floor. The chunk-0 Q
projection is emitted before the K/V setup block so the PE starts as soon
as wq/x0 land (the setup projections wait on the larger wk/wv DMAs), and
warms the HAM clock gate early. PSUM accumulation is fp32 throughout.
"""

import os
import sys

for _p in ("/opt/trn_rl_repo", "/root/.axon_site/_ro/trn_rl_repo"):
    if os.path.isdir(_p) and _p not in sys.path:
        sys.path.insert(0, _p)

import numpy as np
import ml_dtypes

BF16 = ml_dtypes.bfloat16

C = 512        # channels / model dim
T = 4096       # tokens (H*W)
S = 77         # context length
DCTX = 768     # context dim
HEADS = 8
DH = 64        # head dim
NT = 8         # token chunks
TC = T // NT   # 512 tokens per chunk
CT = C // 128  # 4 c-tiles
KT = DCTX // 128  # 6 context-dim tiles
NP = HEADS // 2   # 4 head pairs

_BUILT = None


def _build(dbg=False):
    import concourse.mybir as mybir
    import concourse.tile as tile
    from concourse import bacc
    from concourse.masks import make_identity

    f32 = mybir.dt.float32
    bf16 = mybir.dt.bfloat16
    AF = mybir.ActivationFunctionType

    nc = bacc.Bacc("TRN2", target_bir_lowering=False, debug=False, num_devices=8)

    X = nc.dram_tensor("x", [C, T], bf16, kind="ExternalInput")
    CTX = nc.dram_tensor("ctx", [S, DCTX], bf16, kind="ExternalInput")
    WQ = nc.dram_tensor("wq", [C, C], bf16, kind="ExternalInput")
    WK = nc.dram_tensor("wk", [DCTX, C], bf16, kind="ExternalInput")
    WV = nc.dram_tensor("wv", [DCTX, C], bf16, kind="ExternalInput")
    WO = nc.dram_tensor("wo", [C, C], bf16, kind="ExternalInput")
    BO = nc.dram_tensor("bo", [C], f32, kind="ExternalInput")
    Y = nc.dram_tensor("y", [C, T], bf16, kind="ExternalOutput")

    XR = X[:].rearrange("(o p) t -> p o t", p=128)

    with tile.TileContext(nc) as tc:
        with (
            tc.tile_pool(name="static", bufs=1) as st,
            tc.tile_pool(name="xin", bufs=3) as xp,
            tc.tile_pool(name="qt", bufs=2) as qp,
            tc.tile_pool(name="expsim", bufs=4) as ep,
            tc.tile_pool(name="outut", bufs=2) as op_,
            tc.tile_pool(name="bcast", bufs=3) as bp,
            tc.tile_pool(name="ysb", bufs=4) as yp,
            tc.tile_pool(name="ps_gemm", bufs=2, space="PSUM") as ps_g,
        ):
            # ---- DMA order = first-consumer order: wq + x chunk 0 feed the
            # hoisted chunk-0 Q projection (split in halves so the first
            # matmuls start after ~0.5 MB); ctx/wk/wv feed the K/V setup;
            # wo/bo are first needed one chunk later.
            wq = st.tile([128, CT, C], bf16, tag="wq")
            xs0 = xp.tile([128, CT, TC], bf16, tag="xs")
            for h2 in range(2):
                csl = slice(2 * h2, 2 * h2 + 2)
                nc.sync.dma_start(wq[:, csl, :],
                                  WQ[:].rearrange("(o p) i -> p o i", p=128)[:, csl, :])
                nc.sync.dma_start(xs0[:, csl, :], XR[:, csl, 0:TC])
            ctxs = st.tile([S, DCTX], bf16, tag="ctxs")
            nc.sync.dma_start(ctxs[:], CTX[:])
            wk = st.tile([128, KT, C], bf16, tag="wk")
            nc.sync.dma_start(wk[:], WK[:].rearrange("(o p) i -> p o i", p=128))
            wv = st.tile([128, KT, C], bf16, tag="wv")
            nc.sync.dma_start(wv[:], WV[:].rearrange("(o p) i -> p o i", p=128))
            wo = st.tile([128, CT, C], bf16, tag="wo")
            nc.sync.dma_start(wo[:], WO[:].rearrange("(o p) c -> p o c", p=128))
            bo = st.tile([128, CT], f32, tag="bo")
            nc.sync.dma_start(bo[:], BO[:].rearrange("(o p) -> p o", p=128))

            ident = st.tile([128, 128], bf16, tag="ident")
            make_identity(nc, ident[:])
            # all-ones selector stationaries (bf16): pair-broadcast rowsums.
            # ones_lo[j, c] = (c < 64), ones_hi[j, c] = (c >= 64)
            ones_lo = st.tile([S, 128], bf16, tag="ones_lo")
            nc.gpsimd.memset(ones_lo[:], 0.0)
            nc.gpsimd.memset(ones_lo[:, 0:DH], 1.0)
            ones_hi = st.tile([S, 128], bf16, tag="ones_hi")
            nc.gpsimd.memset(ones_hi[:], 0.0)
            nc.gpsimd.memset(ones_hi[:, DH:128], 1.0)

            # Q projection -> qT [128, 4, TC] (i on partitions); PSUM evac on
            # ACT (DVE carries recip + fused normalize in the main loop).
            def qproj(xs):
                qt = qp.tile([128, CT, TC], bf16, tag="qt")
                for it in range(CT):
                    pq = ps_g.tile([128, TC], f32, tag="pg")
                    for ct in range(CT):
                        nc.tensor.matmul(pq[:], wq[:, ct, it * 128:(it + 1) * 128],
                                         xs[:, ct, :],
                                         start=(ct == 0), stop=(ct == CT - 1))
                    nc.scalar.activation(qt[:, it, :], pq[:], AF.Copy)
                return qt

            # hoisted: chunk-0 Q projection runs while ctx/wk/wv still stream
            qt0 = qproj(xs0)

            # ---- setup: context transpose, K/V projections --------------------
            ctxT = st.tile([128, KT, S], bf16, tag="ctxT")
            ktp = st.tile([128, NP, S], bf16, tag="ktp")    # kT head-pairs
            # vpair[:, p, 0] = [v_2p | 0], vpair[:, p, 1] = [0 | v_2p+1]:
            # zero-padded M=128 stationaries so the AV pair accumulates into
            # one [128, TC] bank without col-tiling (quadrant 3 is invalid).
            vpair = st.tile([S, NP, 2, 128], bf16, tag="vpair")
            nc.gpsimd.memset(vpair[:], 0.0)
            with tc.tile_pool(name="ps_setup", bufs=1, space="PSUM") as ps_st:
                for ct in range(KT):
                    tp = ps_st.tile([128, S], bf16, tag=f"ctx_t{ct % 2}")
                    nc.tensor.transpose(tp[:], ctxs[:, ct * 128:(ct + 1) * 128], ident[0:S, 0:S])
                    nc.vector.tensor_copy(ctxT[:, ct, :], tp[:])
                kps = ps_st.tile([S, C], f32, tag="kproj")
                vps = ps_st.tile([S, C], f32, tag="vproj")
                for ct in range(KT):
                    nc.tensor.matmul(kps[:], ctxT[:, ct, :], wk[:, ct, :],
                                     start=(ct == 0), stop=(ct == KT - 1))
                for ct in range(KT):
                    nc.tensor.matmul(vps[:], ctxT[:, ct, :], wv[:, ct, :],
                                     start=(ct == 0), stop=(ct == KT - 1))
                ksb = st.tile([S, C], bf16, tag="ksb")
                nc.vector.tensor_copy(ksb[:], kps[:])
                for h in range(HEADS):
                    half = h % 2
                    nc.vector.tensor_copy(
                        vpair[:, h // 2, half, half * DH:half * DH + DH],
                        vps[:, h * DH:(h + 1) * DH])
                for h in range(HEADS):
                    tp = ps_st.tile([DH, S], bf16, tag=f"k_t{h % 2}")
                    nc.tensor.transpose(tp[:], ksb[:, h * DH:(h + 1) * DH], ident[0:S, 0:S])
                    base = (h % 2) * DH
                    nc.vector.tensor_copy(ktp[base:base + DH, h // 2, :], tp[:])

            with (
                tc.tile_pool(name="ps_sim", bufs=1, space="PSUM") as ps_sim,
                tc.tile_pool(name="ps_av", bufs=2, space="PSUM") as ps_av,
                tc.tile_pool(name="ps_rs", bufs=2, space="PSUM") as ps_rs,
            ):
                # ---- main loop over token chunks -----------------------------
                def oproj_group(t, ou, ct):
                    tsl = slice(t * TC, (t + 1) * TC)
                    py = ps_g.tile([128, TC], f32, tag="pg")
                    for it in range(CT):
                        nc.tensor.matmul(py[:], wo[:, it, ct * 128:(ct + 1) * 128], ou[:, it, :],
                                         start=(it == 0), stop=(it == CT - 1))
                    ys = yp.tile([128, TC], bf16, tag="ys")
                    if ct % 2 == 0:
                        nc.scalar.activation(ys[:], py[:], AF.Identity, bias=bo[:, ct:ct + 1])
                    else:
                        nc.vector.tensor_scalar_add(ys[:], py[:], bo[:, ct:ct + 1])
                    nc.sync.dma_start(
                        Y[:].rearrange("(o p) t -> p o t", p=128)[:, ct, tsl], ys[:])

                prev = None
                for t in range(NT):
                    if t == 0:
                        xs, qt = xs0, qt0
                    else:
                        xs = xp.tile([128, CT, TC], bf16, tag="xs")
                        nc.sync.dma_start(xs[:], XR[:, :, t * TC:(t + 1) * TC])
                        qt = qproj(xs)

                    # QK^T per pair into one 2-bank psum tile (row-tiled at
                    # bases 0/64: both matmuls run concurrently on the PE),
                    # one paired exp (scale 1/8 fused in ACT); pair-broadcast
                    # rowsums + AV pairs accumulate per pair; chunk t-1's O
                    # projection groups interleave as PE filler.
                    pairs = []
                    ogroups = list(range(CT)) if prev is not None else []

                    def emit_oproj_filler():
                        if ogroups:
                            oproj_group(t - 1, prev[0], ogroups.pop(0))

                    for p in range(NP):
                        psim = ps_sim.tile([128, 2, TC], f32, tag="psim")
                        for half in range(2):
                            base = half * DH
                            nc.tensor.matmul(psim[0:S, half, :], ktp[base:base + DH, p, :],
                                             qt[base:base + DH, p, :])
                        es = ep.tile([S, 2, TC], bf16, tag="exps")
                        nc.scalar.activation(es[:], psim[0:S, :, :], AF.Exp,
                                             scale=DH ** -0.5)
                        pav = ps_av.tile([128, TC], f32, tag="pav")
                        prs = ps_rs.tile([128, TC], f32, tag="prs")
                        pairs.append((pav, prs))
                        for half in range(2):
                            # pair-broadcast rowsum accumulate + AV pair
                            # (zero-padded M=128 stationaries accumulate into
                            # one psum bank)
                            nc.tensor.matmul(prs[:], ones_lo[:] if half == 0 else ones_hi[:],
                                             es[:, half, :], start=(half == 0),
                                             stop=(half == 1))
                            nc.tensor.matmul(pav[:], vpair[:, p, half, :],
                                             es[:, half, :], start=(half == 0),
                                             stop=(half == 1))
                        emit_oproj_filler()

                    # per-pair: bc_p = 1/psrs_p (PSUM -> SBUF, one DVE op),
                    # then fused evac/normalize: ou[:,p,:] = pav_p * bc_p
                    ou = op_.tile([128, CT, TC], bf16, tag="ou")
                    for p in range(NP):
                        pav, prs = pairs[p]
                        bcs = bp.tile([128, TC], f32, tag="bcs")
                        nc.vector.reciprocal_approx_fast(bcs[:], prs[:])
                        nc.vector.tensor_tensor(
                            ou[:, p, :], pav[:], bcs[:],
                            mybir.AluOpType.mult)

                    # leftover O-projection groups for chunk t-1
                    while ogroups:
                        emit_oproj_filler()

                    prev = (ou,)

                # drain: O projection of the last chunk
                for ct in range(CT):
                    oproj_group(NT - 1, prev[0], ct)

    nc.compile()
    return nc


def _get_nc():
    global _BUILT
    if _BUILT is None:
        _BUILT = _build()
    return _BUILT


def kernel(x, context, Wq, Wk, Wv, Wo, bo):
    from concourse.bass_utils import run_bass_kernel_spmd

    B = x.shape[0]
    assert B == 8 and x.shape == (8, C, 64, 64)
    nc = _get_nc()
    x8 = np.asarray(x, dtype=np.float32).reshape(B, C, T).astype(BF16)
    ctx8 = np.asarray(context, dtype=np.float32).astype(BF16)
    wq8 = np.asarray(Wq, np.float32).astype(BF16)
    wk8 = np.asarray(Wk, np.float32).astype(BF16)
    wv8 = np.asarray(Wv, np.float32).astype(BF16)
    wo8 = np.asarray(Wo, np.float32).astype(BF16)
    in_maps = [
        {
            "x": np.ascontiguousarray(x8[b]),
            "ctx": np.ascontiguousarray(ctx8[b]),
            "wq": wq8,
            "wk": wk8,
            "wv": wv8,
            "wo": wo8,
            "bo": np.asarray(bo, np.float32),
        }
        for b in range(8)
    ]
    res = run_bass_kernel_spmd(nc, in_maps, core_ids=list(range(8)))
    return np.stack(
        [np.asarray(r["y"], np.float32).reshape(C, 64, 64) for r in res.results]
    )


# revision 6
# speedup vs baseline: 1.6314x; 1.0122x over previous
"""CrossAttention Trainium2 Bass kernel (v5).

Full inputs in, full output out. Data-parallel over batch: 8 batch elements
-> 8 NeuronCores; each core runs the whole cross-attention for one batch
element. Weights are replicated; no collectives.

Per-core computation (transposed domain end-to-end):
  x [512, 4096] (c-major)  -> qT = Wq.T @ x            [512(i), 4096(t)]
  ctx [77, 768]            -> k/v = ctxT.T @ Wk/Wv     [77(j), 512(i)]
  per head pair p: simT pair in one 2-bank PSUM tile   [77(j), 2, t]
      (the two QK matmuls are row-tiled at partition bases 0/64 and run
      concurrently on the PE); one paired ACT exp evacuates both halves.
  pair-broadcast rowsums: two accumulating matmuls with all-ones selector
      stationaries (ones_lo: cols 0-63, ones_hi: cols 64-127) produce
      psrs_p[c, t] = rowsum_{2p + c//64}[t] directly in broadcast layout.
  bc_p = 1/psrs_p  (DVE reciprocal_approx_fast, PSUM -> SBUF)
  AV pairs: pav_p [128, TC] = zero-padded pair bank (2 accumulating MMs)
  ou_p = pav_p * bc_p  (fused DVE tensor_tensor; PSUM evac + softmax
      normalization in one op; per-pair tiles so the O projection's first
      matmul only waits on pair 0)
  y = Wo.T @ ou + bo                                   [512(c), 4096(t)]

All matmul operands are bf16; inputs are cast AND pre-permuted to the
on-chip [partition, free] layouts host-side, so every DMA moves fully
contiguous >=4KB lines per partition (the v4 strided rearrange reads ran
at ~1/3 DMA rate and delayed the first matmul to 12 us). The output is
written bf16 in a chunk-major layout and unscrambled host-side. A short
burst of scratch matmuls pre-warms the PE's HAM clock gate (cold PE runs
at 1.2 GHz for its first ~3.4 us of activity) while the first DMAs
stream. PSUM accumulation is fp32 throughout.
"""

import os
import sys

for _p in ("/opt/trn_rl_repo", "/root/.axon_site/_ro/trn_rl_repo"):
    if os.path.isdir(_p) and _p not in sys.path:
        sys.path.insert(0, _p)

import numpy as np
import ml_dtypes

BF16 = ml_dtypes.bfloat16

C = 512        # channels / model dim
T = 4096       # tokens (H*W)
S = 77         # context length
DCTX = 768     # context dim
HEADS = 8
DH = 64        # head dim
NT = 8         # token chunks
TC = T // NT   # 512 tokens per chunk
CT = C // 128  # 4 c-tiles
KT = DCTX // 128  # 6 context-dim tiles
NP = HEADS // 2   # 4 head pairs

_BUILT = None


def _build(dbg=False):
    import concourse.mybir as mybir
    import concourse.tile as tile
    from concourse import bacc
    from concourse.masks import make_identity

    f32 = mybir.dt.float32
    bf16 = mybir.dt.bfloat16
    AF = mybir.ActivationFunctionType

    nc = bacc.Bacc("TRN2", target_bir_lowering=False, debug=False, num_devices=8)

    # host-prearranged layouts: partition dim first, contiguous free dims
    X = nc.dram_tensor("x", [NT, 128, CT, TC], bf16, kind="ExternalInput")
    CTX = nc.dram_tensor("ctx", [S, DCTX], bf16, kind="ExternalInput")
    WQ = nc.dram_tensor("wq", [128, CT, C], bf16, kind="ExternalInput")
    WK = nc.dram_tensor("wk", [128, KT, C], bf16, kind="ExternalInput")
    WV = nc.dram_tensor("wv", [128, KT, C], bf16, kind="ExternalInput")
    WO = nc.dram_tensor("wo", [128, CT, C], bf16, kind="ExternalInput")
    BO = nc.dram_tensor("bo", [128, CT], f32, kind="ExternalInput")
    Y = nc.dram_tensor("y", [128, CT, NT, TC], bf16, kind="ExternalOutput")

    XR = X[:].rearrange("n p c t -> p n c t")

    with tile.TileContext(nc) as tc:
        with (
            tc.tile_pool(name="static", bufs=1) as st,
            tc.tile_pool(name="xin", bufs=3) as xp,
            tc.tile_pool(name="qt", bufs=2) as qp,
            tc.tile_pool(name="expsim", bufs=4) as ep,
            tc.tile_pool(name="outut", bufs=8) as op_,
            tc.tile_pool(name="bcast", bufs=3) as bp,
            tc.tile_pool(name="ysb", bufs=4) as yp,
            tc.tile_pool(name="ps_gemm", bufs=2, space="PSUM") as ps_g,
        ):
            # ---- DMA order = first-consumer order: wq + x chunk 0 feed the
            # hoisted chunk-0 Q projection (split in halves so the first
            # matmuls start after ~0.5 MB); ctx/wk/wv feed the K/V setup;
            # wo/bo are first needed one chunk later.
            wq = st.tile([128, CT, C], bf16, tag="wq")
            xs0 = xp.tile([128, CT, TC], bf16, tag="xs")
            for h2 in range(2):
                csl = slice(2 * h2, 2 * h2 + 2)
                nc.sync.dma_start(wq[:, csl, :], WQ[:][:, csl, :])
                nc.sync.dma_start(xs0[:, csl, :], XR[:, 0, csl, :])
            ctxs = st.tile([S, DCTX], bf16, tag="ctxs")
            nc.sync.dma_start(ctxs[:], CTX[:])
            wk = st.tile([128, KT, C], bf16, tag="wk")
            nc.sync.dma_start(wk[:], WK[:])
            wv = st.tile([128, KT, C], bf16, tag="wv")
            nc.sync.dma_start(wv[:], WV[:])
            wo = st.tile([128, CT, C], bf16, tag="wo")
            nc.sync.dma_start(wo[:], WO[:])
            bo = st.tile([128, CT], f32, tag="bo")
            nc.sync.dma_start(bo[:], BO[:])

            ident = st.tile([128, 128], bf16, tag="ident")
            make_identity(nc, ident[:])
            # all-ones selector stationaries (bf16): pair-broadcast rowsums.
            # ones_lo[j, c] = (c < 64), ones_hi[j, c] = (c >= 64)
            ones_lo = st.tile([S, 128], bf16, tag="ones_lo")
            nc.gpsimd.memset(ones_lo[:], 0.0)
            nc.gpsimd.memset(ones_lo[:, 0:DH], 1.0)
            ones_hi = st.tile([S, 128], bf16, tag="ones_hi")
            nc.gpsimd.memset(ones_hi[:], 0.0)
            nc.gpsimd.memset(ones_hi[:, DH:128], 1.0)

            # ---- PE pre-warm: ~10 scratch matmuls (~4.3 us at the cold 1.2
            # GHz clock) while the first DMAs stream. HAM un-throttles to 2.4
            # GHz after ~3.4 us of sustained PE activity, so the real matmuls
            # start warm. No data dependencies: operates on a memset tile.
            scr = st.tile([128, TC], bf16, tag="scr")
            nc.vector.memset(scr[:], 0.0)
            with tc.tile_pool(name="ps_warm", bufs=1, space="PSUM") as ps_w:
                pw = ps_w.tile([128, TC], f32, tag="pw")
                for _ in range(10):
                    nc.tensor.matmul(pw[:], scr[:, 0:128], scr[:])

            # Q projection -> qT [128, 4, TC] (i on partitions); PSUM evac on
            # ACT (DVE carries recip + fused normalize in the main loop).
            def qproj(xs):
                qt = qp.tile([128, CT, TC], bf16, tag="qt")
                for it in range(CT):
                    pq = ps_g.tile([128, TC], f32, tag="pg")
                    for ct in range(CT):
                        nc.tensor.matmul(pq[:], wq[:, ct, it * 128:(it + 1) * 128],
                                         xs[:, ct, :],
                                         start=(ct == 0), stop=(ct == CT - 1))
                    nc.scalar.activation(qt[:, it, :], pq[:], AF.Copy)
                return qt

            # hoisted: chunk-0 Q projection runs while ctx/wk/wv still stream
            qt0 = qproj(xs0)

            # ---- setup: context transpose, K/V projections --------------------
            ctxT = st.tile([128, KT, S], bf16, tag="ctxT")
            ktp = st.tile([128, NP, S], bf16, tag="ktp")    # kT head-pairs
            # vpair[:, p, 0] = [v_2p | 0], vpair[:, p, 1] = [0 | v_2p+1]:
            # zero-padded M=128 stationaries so the AV pair accumulates into
            # one [128, TC] bank without col-tiling (quadrant 3 is invalid).
            vpair = st.tile([S, NP, 2, 128], bf16, tag="vpair")
            nc.gpsimd.memset(vpair[:], 0.0)
            with tc.tile_pool(name="ps_setup", bufs=1, space="PSUM") as ps_st:
                for ct in range(KT):
                    tp = ps_st.tile([128, S], bf16, tag=f"ctx_t{ct % 2}")
                    nc.tensor.transpose(tp[:], ctxs[:, ct * 128:(ct + 1) * 128], ident[0:S, 0:S])
                    nc.vector.tensor_copy(ctxT[:, ct, :], tp[:])
                kps = ps_st.tile([S, C], f32, tag="kproj")
                vps = ps_st.tile([S, C], f32, tag="vproj")
                for ct in range(KT):
                    nc.tensor.matmul(kps[:], ctxT[:, ct, :], wk[:, ct, :],
                                     start=(ct == 0), stop=(ct == KT - 1))
                for ct in range(KT):
                    nc.tensor.matmul(vps[:], ctxT[:, ct, :], wv[:, ct, :],
                                     start=(ct == 0), stop=(ct == KT - 1))
                ksb = st.tile([S, C], bf16, tag="ksb")
                nc.vector.tensor_copy(ksb[:], kps[:])
                for h in range(HEADS):
                    half = h % 2
                    nc.vector.tensor_copy(
                        vpair[:, h // 2, half, half * DH:half * DH + DH],
                        vps[:, h * DH:(h + 1) * DH])
                for h in range(HEADS):
                    tp = ps_st.tile([DH, S], bf16, tag=f"k_t{h % 2}")
                    nc.tensor.transpose(tp[:], ksb[:, h * DH:(h + 1) * DH], ident[0:S, 0:S])
                    base = (h % 2) * DH
                    nc.vector.tensor_copy(ktp[base:base + DH, h // 2, :], tp[:])

            with (
                tc.tile_pool(name="ps_sim", bufs=1, space="PSUM") as ps_sim,
                tc.tile_pool(name="ps_av", bufs=2, space="PSUM") as ps_av,
                tc.tile_pool(name="ps_rs", bufs=2, space="PSUM") as ps_rs,
            ):
                # ---- main loop over token chunks -----------------------------
                def oproj_group(t, ou, ct):
                    py = ps_g.tile([128, TC], f32, tag="pg")
                    for it in range(CT):
                        nc.tensor.matmul(py[:], wo[:, it, ct * 128:(ct + 1) * 128], ou[it][:],
                                         start=(it == 0), stop=(it == CT - 1))
                    ys = yp.tile([128, TC], bf16, tag="ys")
                    if ct % 2 == 0:
                        nc.scalar.activation(ys[:], py[:], AF.Identity, bias=bo[:, ct:ct + 1])
                    else:
                        nc.vector.tensor_scalar_add(ys[:], py[:], bo[:, ct:ct + 1])
                    nc.sync.dma_start(Y[:][:, ct, t, :], ys[:])

                prev = None
                for t in range(NT):
                    if t == 0:
                        xs, qt = xs0, qt0
                    else:
                        xs = xp.tile([128, CT, TC], bf16, tag="xs")
                        nc.sync.dma_start(xs[:], XR[:, t, :, :])
                        qt = qproj(xs)

                    # QK^T per pair into one 2-bank psum tile (row-tiled at
                    # bases 0/64: both matmuls run concurrently on the PE),
                    # one paired exp (scale 1/8 fused in ACT); pair-broadcast
                    # rowsums + AV pairs accumulate per pair; chunk t-1's O
                    # projection groups interleave as PE filler.
                    pairs = []
                    ogroups = list(range(CT)) if prev is not None else []

                    def emit_oproj_filler():
                        if ogroups:
                            oproj_group(t - 1, prev, ogroups.pop(0))

                    for p in range(NP):
                        psim = ps_sim.tile([128, 2, TC], f32, tag="psim")
                        for half in range(2):
                            base = half * DH
                            nc.tensor.matmul(psim[0:S, half, :], ktp[base:base + DH, p, :],
                                             qt[base:base + DH, p, :])
                        es = ep.tile([S, 2, TC], bf16, tag="exps")
                        nc.scalar.activation(es[:], psim[0:S, :, :], AF.Exp,
                                             scale=DH ** -0.5)
                        pav = ps_av.tile([128, TC], f32, tag="pav")
                        prs = ps_rs.tile([128, TC], f32, tag="prs")
                        pairs.append((pav, prs))
                        for half in range(2):
                            # pair-broadcast rowsum accumulate + AV pair
                            # (zero-padded M=128 stationaries accumulate into
                            # one psum bank)
                            nc.tensor.matmul(prs[:], ones_lo[:] if half == 0 else ones_hi[:],
                                             es[:, half, :], start=(half == 0),
                                             stop=(half == 1))
                            nc.tensor.matmul(pav[:], vpair[:, p, half, :],
                                             es[:, half, :], start=(half == 0),
                                             stop=(half == 1))
                        emit_oproj_filler()

                    # per-pair: bc_p = 1/psrs_p (PSUM -> SBUF, one DVE op),
                    # then fused evac/normalize: ou_p = pav_p * bc_p
                    ou = []
                    for p in range(NP):
                        pav, prs = pairs[p]
                        bcs = bp.tile([128, TC], f32, tag="bcs")
                        nc.vector.reciprocal_approx_fast(bcs[:], prs[:])
                        oup = op_.tile([128, TC], bf16, tag="ou")
                        nc.vector.tensor_tensor(
                            oup[:], pav[:], bcs[:],
                            mybir.AluOpType.mult)
                        ou.append(oup)

                    # leftover O-projection groups for chunk t-1
                    while ogroups:
                        emit_oproj_filler()

                    prev = ou

                # drain: O projection of the last chunk
                for ct in range(CT):
                    oproj_group(NT - 1, prev, ct)

    nc.compile()
    return nc


def _get_nc():
    global _BUILT
    if _BUILT is None:
        _BUILT = _build()
    return _BUILT


def kernel(x, context, Wq, Wk, Wv, Wo, bo):
    from concourse.bass_utils import run_bass_kernel_spmd

    B = x.shape[0]
    assert B == 8 and x.shape == (8, C, 64, 64)
    nc = _get_nc()

    def wlayout(w, kt):  # [K, C] -> [128, kt, C] with row k = o*128 + p
        return np.ascontiguousarray(
            np.asarray(w, np.float32).astype(BF16).reshape(kt, 128, C).transpose(1, 0, 2))

    # x[b]: [C, T] -> [NT, 128, CT, TC] with c = ct*128 + p, t = n*TC + tc
    x8 = (np.asarray(x, np.float32).reshape(B, CT, 128, NT, TC)
          .transpose(0, 3, 2, 1, 4).astype(BF16))
    ctx8 = np.asarray(context, dtype=np.float32).astype(BF16)
    wq8 = wlayout(Wq, CT)
    wk8 = wlayout(Wk, KT)
    wv8 = wlayout(Wv, KT)
    wo8 = wlayout(Wo, CT)
    bo8 = np.ascontiguousarray(np.asarray(bo, np.float32).reshape(CT, 128).T)
    in_maps = [
        {
            "x": np.ascontiguousarray(x8[b]),
            "ctx": np.ascontiguousarray(ctx8[b]),
            "wq": wq8,
            "wk": wk8,
            "wv": wv8,
            "wo": wo8,
            "bo": bo8,
        }
        for b in range(8)
    ]
    res = run_bass_kernel_spmd(nc, in_maps, core_ids=list(range(8)))
    # y dram [128, CT, NT, TC] -> [C, T] with c = ct*128 + p
    out = []
    for r in res.results:
        yb = np.asarray(r["y"], np.float32)  # [128, CT, NT, TC]
        out.append(yb.transpose(1, 0, 2, 3).reshape(C, T).reshape(C, 64, 64))
    return np.stack(out)
